# revision 1
# baseline (speedup 1.0000x reference)
"""ConfidenceGate Trainium2 kernel (8 NeuronCores, SPMD).

Problem recap (shapes hardcoded from the spec):
  x:      (4, 512, 256, 7, 7) f32
  prev_x: (4, 512, 256, 7, 7) f32
  match:  (4, 512, 513) f32
  + tiny proj/LN/MLP params.
Reference returns c[0] -> (512, 1): only batch 0 contributes to the output.

Strategy:
  * Only batch 0 is computed (the reference discards batches 1..3).
  * Data-parallel over M=512 ROI rows: 8 cores x 64 rows.
  * The gather prev_pool[top1] indexes within batch row 0 only.  top1 =
    argmax(match[0,:,:512]) is computed on host (cheap: 1 MB argmax) and used
    to pre-gather the raw prev_x rows per shard, so every core reads just its
    own 64 rows of x and 64 gathered rows of prev_x (pooling commutes with
    the gather, exactly as the reference notes).
  * On device per core: spatial mean-pool (the memory-bound part, 6.4 MB),
    match stats (mass/top2/entropy), proj matmul + layernorm, cosine
    similarity, 5->32->1 MLP gate, sigmoid + mask + clip.

Perf notes (per trace analysis):
  * Big loads stream on the sync HWDGE ring in chunks; per-chunk pooling
    reduce (DVE) -> PE band transpose -> scaled deinterleave (ACT) -> K=32
    proj matmul accumulation keeps everything off the critical tail.
  * Small loads (match shard + one packed aux tensor) ride the scalar HWDGE
    ring so they don't queue behind the 6.4 MB stream.
  * ACT tables (Ln/Sqrt/Sigmoid) preloaded via dummy activations.
  * MLP runs transposed ((32,64)/(1,64) tiles) so b1/b2 are per-partition
    activation biases and the output DMA is one contiguous 256 B descriptor.
"""

import sys

if "/opt/trn_rl_repo" not in sys.path:
    sys.path.insert(0, "/opt/trn_rl_repo")

import numpy as np

B, M, N, C, G = 4, 512, 512, 256, 7
S = G * G                      # 49 spatial positions
PP, HH = 32, 32                # proj dim, MLP hidden
NCORES = 8
MS = M // NCORES               # 64 rows per core
ROW = C * S                    # 12544 elements per ROI row
HALF = ROW // 2                # 6272 = 128 channels * 49

# chunk sizes (in free elements of the (128, 6272) view); multiples of 49
XCH = [1568, 1568, 1568, 1568]
VCH = [1568, 1568, 1568, 784, 784]

# channel bands (offset, width) used by the streamed proj accumulation;
# pw is stored band-major in aux so every matmul operand is partition-base-0
BANDS = [(0, 32), (32, 32), (64, 32), (96, 32), (96, 16), (112, 16)]
BAND_IDX = {b: i for i, b in enumerate(BANDS)}

# aux tensor column layout
A_PWB = 0       # band-major pw: band i at cols [64*i : 64*i+64], rows 0:width
A_ID = 384      # identity (128, 128)
A_PB = 512      # proj_b row-replicated (64, 32)
A_LG = 544      # ln_g row-replicated (64, 32)
A_LB = 576      # ln_b row-replicated (64, 32)
A_B1 = 608      # b1 as column (32, 1)
A_W2 = 609      # w2[0] as column (32, 1)
A_B2 = 610      # b2 (1, 1)
A_W1 = 611      # w1.T (5, 32)
A_COLS = 643

EPS = 1e-9
LN_EPS = 1e-5
NORM_EPS = 1e-12

_CACHE = {}


def _build():
    import concourse.bacc as bacc
    import concourse.tile as tile
    import concourse.mybir as mybir

    dt = mybir.dt
    Alu = mybir.AluOpType
    Act = mybir.ActivationFunctionType
    Ax = mybir.AxisListType
    f32 = dt.float32

    nc = bacc.Bacc("TRN2", target_bir_lowering=False, debug=False)

    xs_d = nc.dram_tensor("xs", [128, HALF], f32, kind="ExternalInput")
    pv_d = nc.dram_tensor("pv", [128, HALF], f32, kind="ExternalInput")
    mt_d = nc.dram_tensor("mt", [MS, N + 1], f32, kind="ExternalInput")
    aux_d = nc.dram_tensor("aux", [128, A_COLS], f32, kind="ExternalInput")
    out_d = nc.dram_tensor("out", [1, MS], f32, kind="ExternalOutput")

    with tile.TileContext(nc) as tc:
        with (
            tc.tile_pool(name="persist", bufs=1) as per,
            tc.tile_pool(name="chunks", bufs=1) as big,
            tc.tile_pool(name="scratch", bufs=1) as scr,
            tc.tile_pool(name="scrbig", bufs=2) as scrb,
            tc.tile_pool(name="psum", bufs=1, space="PSUM") as psp,
            tc.tile_pool(name="psband", bufs=2, space="PSUM") as psb,
        ):
            # ---- small loads on the scalar (ACT) HWDGE ring ----
            mt = per.tile([MS, N + 1], f32)
            nc.scalar.dma_start(out=mt[:], in_=mt_d[:])
            aux = per.tile([128, A_COLS], f32)
            nc.scalar.dma_start(out=aux[:], in_=aux_d[:])

            # ---- big chunked loads on the sync HWDGE ring, x/v interleaved --
            seq = []   # (which, j, tile, foff, flen, coff, clen)
            xoff = [0]
            for w in XCH:
                xoff.append(xoff[-1] + w)
            voff = [0]
            for w in VCH:
                voff.append(voff[-1] + w)
            order = []
            for j in range(max(len(XCH), len(VCH))):
                if j < len(XCH):
                    order.append(("x", j))
                if j < len(VCH):
                    order.append(("v", j))
            # append leftover v chunks (VCH longer)
            for which, j in order:
                src, offs, widths = (
                    (xs_d, xoff, XCH) if which == "x" else (pv_d, voff, VCH))
                fo, fl = offs[j], widths[j]
                ct = big.tile([128, fl], f32, tag=f"ch_{which}{j}", name=f"ch_{which}{j}")
                nc.sync.dma_start(out=ct[:], in_=src[:, fo:fo + fl])
                seq.append((which, j, ct, fo, fl, fo // S, fl // S))

            # ---- constants / ACT table preloads ----
            e9 = per.tile([MS, 1], f32)
            nc.gpsimd.memset(e9[:], EPS)
            eln = per.tile([MS, 1], f32)
            nc.gpsimd.memset(eln[:], LN_EPS)
            dmy = per.tile([1, 1], f32)
            nc.gpsimd.memset(dmy[:], 1.0)
            pre = scr.tile([1, 1], f32, tag="pre")
            nc.scalar.activation(pre[:], dmy[:], Act.Ln, bias=e9[0:1, 0:1])
            pre2 = scr.tile([1, 1], f32, tag="pre")
            nc.scalar.activation(pre2[:], dmy[:], Act.Sqrt, bias=eln[0:1, 0:1])
            pre3 = scr.tile([1, 1], f32, tag="pre")
            nc.scalar.activation(pre3[:], dmy[:], Act.Sigmoid, bias=e9[0:1, 0:1])

            real = mt[:, 0:N]
            pd = mt[:, N:N + 1]
            feat = per.tile([MS, 6], f32)

            # ---- match stats ----
            # rmass via ACT accumulator (frees DVE)
            rmass = per.tile([MS, 1], f32)
            jr = scrb.tile([MS, N], f32, tag="jk")
            nc.scalar.activation(jr[:], real, Act.Copy, accum_out=rmass[:])
            # ln(real + 1e-9) on ACT
            lnr = per.tile([MS, N], f32)
            nc.scalar.activation(lnr[:], real, Act.Ln, bias=e9[:])
            # p_max -> feat[:,2]
            nc.vector.reduce_max(feat[:, 2:3], real, axis=Ax.X)
            # mask out the max, re-reduce for second max
            eqm = scrb.tile([MS, N], f32, tag="jk")
            nc.vector.tensor_scalar(eqm[:], real, feat[:, 2:3], None, op0=Alu.is_equal)
            msk = scrb.tile([MS, N], f32, tag="jk")
            nc.vector.scalar_tensor_tensor(
                msk[:], eqm[:], -3.4e38, real, op0=Alu.mult, op1=Alu.add)
            m2 = per.tile([MS, 1], f32)
            nc.vector.reduce_max(m2[:], msk[:], axis=Ax.X)
            nc.vector.tensor_tensor(feat[:, 3:4], feat[:, 2:3], m2[:], op=Alu.subtract)
            # feat[:,3] = sum(real * ln(real+eps)) = -entropy (matches ref to ~1e-7)
            je = scrb.tile([MS, N], f32, tag="jk")
            nc.vector.scalar_tensor_tensor(
                je[:], real, 1.0, lnr[:],
                op0=Alu.mult, op1=Alu.mult, accum_out=feat[:, 4:5])
            # feat[:,0] = 1 - p_dummy
            nc.vector.tensor_scalar(feat[:, 1:2], pd, -1.0, 1.0, op0=Alu.mult, op1=Alu.add)
            # masks: hr9 (cos gate), hr6 (output gate) -> feat[:,5]
            hr9 = per.tile([MS, 1], f32)
            nc.vector.tensor_scalar(hr9[:], rmass[:], EPS, None, op0=Alu.is_gt)
            nc.vector.tensor_scalar(feat[:, 0:1], rmass[:], 1e-6, None, op0=Alu.is_gt)

            # ---- proj psum tiles, preloaded with proj_b (matmuls accumulate) --
            vps = {}
            for w in ("x", "v"):
                t = psp.tile([MS, PP], f32, tag=f"vps_{w}", name=f"vps_{w}")
                nc.scalar.activation(t[:], aux[0:MS, A_PB:A_PB + PP], Act.Copy)
                vps[w] = t

            # ---- streamed pooling + band transpose + proj accumulation ----
            P_t = {"x": per.tile([128, 128], f32, tag="P_x", name="P_x"),
                   "v": per.tile([128, 128], f32, tag="P_v", name="P_v")}
            iden = aux[:, A_ID:A_ID + 128]
            nbands = {"x": len(XCH), "v": len(VCH)}
            for which, j, ct, fo, fl, co, cl in seq:
                P = P_t[which]
                nc.vector.reduce_sum(
                    P[:, co:co + cl],
                    ct[:].rearrange("p (c s) -> p c s", s=S), axis=Ax.X)
                ps = psb.tile([cl, 128], f32, tag=f"band{len(seq) % 2}",
                              name=f"ps_{which}{j}")
                nc.tensor.transpose(ps[:], P[:, co:co + cl], iden)
                sb = scr.tile([cl, 128], f32, tag=f"sb_{which}{j % 2}",
                              name=f"sb_{which}{j}")
                for h in range(2):
                    nc.scalar.activation(
                        sb[:, h * 64:(h + 1) * 64], ps[:, h::2],
                        Act.Copy, scale=1.0 / S)
                last = j == nbands[which] - 1
                pwb = A_PWB + 64 * BAND_IDX[(co, cl)]
                for h in range(2):
                    nc.tensor.matmul(
                        vps[which][:],
                        sb[:, h * 64:(h + 1) * 64],
                        aux[0:cl, pwb + h * PP:pwb + (h + 1) * PP],
                        start=False, stop=last and h == 1,
                        skip_group_check=True)

            # ---- layernorm per vec (ACT-heavy to keep DVE clear) ----
            ys = {}
            for w in ("x", "v"):
                vp = vps[w]
                msum = scr.tile([MS, 1], f32, tag=f"ms_{w}")
                jm = scr.tile([MS, PP], f32, tag=f"jm_{w}")
                nc.scalar.activation(jm[:], vp[:], Act.Copy, accum_out=msum[:])
                mmean = scr.tile([MS, 1], f32, tag=f"mm_{w}")
                nc.scalar.activation(mmean[:], msum[:], Act.Copy, scale=1.0 / PP)
                ctr = scr.tile([MS, PP], f32, tag=f"ctr_{w}")
                nc.vector.tensor_scalar_sub(ctr[:], vp[:], mmean[:])
                sq = scr.tile([MS, PP], f32, tag=f"sq_{w}")
                vsum = scr.tile([MS, 1], f32, tag=f"vs_{w}")
                nc.scalar.activation(sq[:], ctr[:], Act.Square, accum_out=vsum[:])
                den = scr.tile([MS, 1], f32, tag=f"dn_{w}")
                nc.scalar.activation(den[:], vsum[:], Act.Sqrt, scale=1.0 / PP, bias=eln[:])
                rden = scr.tile([MS, 1], f32, tag=f"rd_{w}")
                nc.vector.reciprocal(rden[:], den[:])
                y = scr.tile([MS, PP], f32, tag=f"y_{w}")
                nc.vector.scalar_tensor_tensor(
                    y[:], ctr[:], rden[:], aux[0:MS, A_LG:A_LG + PP],
                    op0=Alu.mult, op1=Alu.mult)
                y2 = per.tile([MS, PP], f32, tag=f"y2_{w}")
                nc.vector.tensor_tensor(y2[:], y[:], aux[0:MS, A_LB:A_LB + PP], op=Alu.add)
                ys[w] = y2

            # ---- cosine similarity -> feat[:,4] ----
            yx, yv = ys["x"], ys["v"]
            dot = per.tile([MS, 1], f32)
            jc = scr.tile([MS, PP], f32, tag="jc")
            nc.vector.scalar_tensor_tensor(
                jc[:], yx[:], 1.0, yv[:], op0=Alu.mult, op1=Alu.mult, accum_out=dot[:])
            nrm2 = per.tile([MS, 2], f32)
            jn = scr.tile([MS, PP], f32, tag="jc")
            nc.scalar.activation(jn[:], yx[:], Act.Square, accum_out=nrm2[:, 0:1])
            jn2 = scr.tile([MS, PP], f32, tag="jc")
            nc.scalar.activation(jn2[:], yv[:], Act.Square, accum_out=nrm2[:, 1:2])
            nrm = per.tile([MS, 2], f32)
            nc.scalar.activation(nrm[:], nrm2[:], Act.Sqrt)
            nc.vector.tensor_scalar_max(nrm[:], nrm[:], NORM_EPS)
            dn2 = per.tile([MS, 1], f32)
            nc.vector.tensor_tensor(dn2[:], nrm[:, 0:1], nrm[:, 1:2], op=Alu.mult)
            rdn = per.tile([MS, 1], f32)
            nc.vector.reciprocal(rdn[:], dn2[:])
            nc.vector.scalar_tensor_tensor(
                feat[:, 5:6], dot[:], rdn[:], hr9[:], op0=Alu.mult, op1=Alu.mult)

            # ---- MLP gate, transposed layout ----
            fT = psp.tile([6, MS], f32, tag="fT")
            nc.tensor.transpose(fT[:], feat[:], aux[0:MS, A_ID:A_ID + MS])
            fTs = per.tile([6, MS], f32)
            nc.scalar.activation(fTs[:], fT[:], Act.Copy)
            hps = psp.tile([HH, MS], f32, tag="hps")
            nc.tensor.matmul(hps[:], aux[0:6, A_W1:A_W1 + HH], fTs[0:6, :],
                             start=True, stop=True)
            reluT = per.tile([HH, MS], f32)
            nc.scalar.activation(reluT[:], hps[:], Act.Relu, bias=aux[0:HH, A_B1:A_B1 + 1])
            lps = psp.tile([1, MS], f32, tag="lps")
            nc.tensor.matmul(lps[:], aux[0:HH, A_W2:A_W2 + 1], reluT[:],
                             start=True, stop=True)
            sg = per.tile([1, MS], f32)
            nc.scalar.activation(sg[:], lps[:], Act.Sigmoid, bias=aux[0:1, A_B2:A_B2 + 1])
            gt = per.tile([1, MS], f32)
            nc.vector.tensor_tensor(gt[:], sg[:], fTs[0:1, :], op=Alu.mult)
            res = per.tile([1, MS], f32)
            nc.vector.tensor_scalar(res[:], gt[:], 0.001, 0.999, op0=Alu.max, op1=Alu.min)
            nc.sync.dma_start(out=out_d[:], in_=res[:])

    nc.finalize()
    return nc


def _get_nc():
    if "nc" not in _CACHE:
        _CACHE["nc"] = _build()
    return _CACHE["nc"]


def make_in_maps(x, prev_x, match, proj_w, proj_b, ln_g, ln_b, w1, b1, w2, b2):
    f32 = np.float32
    x0 = np.asarray(x[0], dtype=f32)
    p0 = np.asarray(prev_x[0], dtype=f32)
    mt0 = np.ascontiguousarray(np.asarray(match[0], dtype=f32))
    real0 = mt0[:, :N]
    rm = real0.sum(axis=1)
    top1 = np.where(rm > EPS, np.argmax(real0, axis=1), 0)

    proj_w = np.asarray(proj_w, dtype=f32)
    pw_packed = (
        proj_w.T.reshape(2, 128, PP).transpose(1, 0, 2).reshape(128, 2 * PP))
    aux = np.zeros((128, A_COLS), dtype=f32)
    for i, (co, cl) in enumerate(BANDS):
        aux[0:cl, A_PWB + 64 * i:A_PWB + 64 * i + 64] = pw_packed[co:co + cl, :]
    aux[:, A_ID:A_ID + 128] = np.eye(128, dtype=f32)
    aux[0:MS, A_PB:A_PB + PP] = np.asarray(proj_b, dtype=f32)
    aux[0:MS, A_LG:A_LG + PP] = np.asarray(ln_g, dtype=f32)
    aux[0:MS, A_LB:A_LB + PP] = np.asarray(ln_b, dtype=f32)
    aux[0:HH, A_B1] = np.asarray(b1, dtype=f32)
    aux[0:HH, A_W2] = np.asarray(w2, dtype=f32)[0]
    aux[0:1, A_B2] = np.asarray(b2, dtype=f32)[0]
    aux[1:6, A_W1:A_W1 + HH] = np.asarray(w1, dtype=f32).T

    in_maps = []
    for i in range(NCORES):
        lo, hi = i * MS, (i + 1) * MS
        xs = np.ascontiguousarray(x0[lo:hi]).reshape(128, HALF)
        pv = np.ascontiguousarray(p0[top1[lo:hi]]).reshape(128, HALF)
        in_maps.append({
            "xs": xs, "pv": pv, "mt": np.ascontiguousarray(mt0[lo:hi]),
            "aux": aux,
        })
    return in_maps


def run(in_maps, trace=False):
    from concourse.bass_utils import run_bass_kernel_spmd
    res = run_bass_kernel_spmd(_get_nc(), in_maps, list(range(NCORES)), trace=trace)
    out = np.concatenate(
        [res.results[i]["out"].reshape(MS, 1) for i in range(NCORES)], axis=0)
    return out.astype(np.float32), res


def kernel(x, prev_x, match, proj_w, proj_b, ln_g, ln_b, w1, b1, w2, b2):
    in_maps = make_in_maps(x, prev_x, match, proj_w, proj_b, ln_g, ln_b, w1, b1, w2, b2)
    out, _ = run(in_maps, trace=False)
    return out

